# revision 6
# baseline (speedup 1.0000x reference)
"""Trainium2 kernel for nn_CMSBlockLinear (block-sparse linear layer).

Strategy: the 50%-dense random block topology (16x16 blocks) is hostile to
the 128x128 PE array, so densify the weights host-side (pure index
plumbing, no FLOPs) and run a dense [8192,2048]x[2048,8192] matmul,
token-sharded 8 ways across NeuronCores. All matmul operands use fp32r
(fp32 with 15-bit mantissa) which streams at full PE rate for N>=256.

Per core: out[1024 tok, 8192 feat] = xT_shard.T @ W_dense
  - xT shard [128, 16, 1024] fp32r lives in SBUF; chunks are the
    stationary matmul operand.
  - W streamed in [128, 512] tiles (contiguous 256 KB DMAs), the moving
    operand.
  - psum [128 tok, 512 feat] accumulates over the 16 contraction chunks;
    output is written in [tokens, features] layout directly, so the host
    just concatenates the 8 shards.
"""

import sys

sys.path.insert(0, "/opt/trn_rl_repo")

import numpy as np

T, IN_F, OUT_F = 8192, 2048, 8192
NCORES = 8
TPC = T // NCORES  # 1024 tokens per core
KO = IN_F // 128  # 16 contraction chunks of 128
NT = OUT_F // 512  # 16 feature tiles of 512
MT = TPC // 128  # 8 token tiles of 128

_cached_nc = None


def _round_f32r(a: np.ndarray) -> np.ndarray:
    """Round fp32 to fp32r (15-bit mantissa) with round-to-nearest-even."""
    b = np.ascontiguousarray(a, dtype=np.float32).view(np.uint32)
    r = (b + np.uint32(0x7F) + ((b >> np.uint32(8)) & np.uint32(1))) & np.uint32(
        0xFFFFFF00
    )
    return r.view(np.float32)


def _build_program():
    global _cached_nc
    if _cached_nc is not None:
        return _cached_nc
    from concourse import bacc, mybir, tile

    F32, F32R = mybir.dt.float32, mybir.dt.float32r

    nc = bacc.Bacc(None)
    xT = nc.declare_dram_parameter("xT", [128, KO, TPC], F32R, isOutput=False)
    W = nc.declare_dram_parameter("W", [NT, KO, 128, 512], F32R, isOutput=False)
    out = nc.declare_dram_parameter("out", [TPC, OUT_F], F32, isOutput=True)

    with tile.TileContext(nc) as tc:
        with tc.tile_pool(name="xt", bufs=1) as xpool, \
             tc.tile_pool(name="wt", bufs=12) as wpool, \
             tc.tile_pool(name="ot", bufs=12) as opool, \
             tc.tile_pool(name="ps", bufs=1, space="PSUM") as ps:
            xts = []
            for ko in range(KO):
                # scalar engine's hardware-DGE queue: parallel with the W
                # stream on sync (so the first matmuls aren't stuck behind
                # the 8MB xT load), and unlike gpsimd's software queue its
                # completion semaphores tick promptly. The queue is free
                # until output evictions begin.
                xk = xpool.tile([128, TPC], F32R, tag=f"x{ko}", name=f"xk{ko}")
                nc.scalar.dma_start(out=xk[:], in_=xT[:, ko, :])
                xts.append(xk)
            for n in range(NT):
                psums = [
                    ps.tile([128, 512], F32, tag=f"p{m}", name=f"ps{n}_{m}")
                    for m in range(MT)
                ]
                for ko in range(KO):
                    wt = wpool.tile([128, 512], F32R, tag="w", name=f"w{n}_{ko}")
                    nc.sync.dma_start(out=wt[:], in_=W[n, ko])
                    for m in range(MT):
                        nc.tensor.matmul(
                            psums[m][:],
                            xts[ko][:, m * 128 : (m + 1) * 128],
                            wt[:],
                            start=(ko == 0),
                            stop=(ko == KO - 1),
                        )
                for m in range(MT):
                    ot = opool.tile([128, 512], F32, tag="o", name=f"o{n}_{m}")
                    nc.vector.tensor_copy(ot[:], psums[m][:])
                    nc.scalar.dma_start(
                        out=out[m * 128 : (m + 1) * 128, n * 512 : (n + 1) * 512],
                        in_=ot[:],
                    )
    nc.compile()
    _cached_nc = nc
    return nc


def _prep_inputs(x, values, bias, col_indices):
    x = np.ascontiguousarray(np.asarray(x), dtype=np.float32)
    values = np.ascontiguousarray(np.asarray(values), dtype=np.float32)
    bias = np.asarray(bias, dtype=np.float32)
    col_indices = np.asarray(col_indices, dtype=np.int32)

    R, K = col_indices.shape  # 512, 64
    C = IN_F // 16  # 128 column blocks

    # Scatter block values into the dense weight matrix Wd[k_in, n_out].
    Wb = np.zeros((C, R, 16, 16), np.float32)  # [c, r, i, o]
    r_idx = np.broadcast_to(np.arange(R, dtype=np.int64)[:, None], col_indices.shape)
    Wb[col_indices, r_idx] = values.transpose(0, 1, 3, 2)  # values[r,k,o,i] -> [i,o]
    Wd = Wb.transpose(0, 2, 1, 3).reshape(IN_F, OUT_F)
    Wd = _round_f32r(Wd)
    W4 = np.ascontiguousarray(
        Wd.reshape(KO, 128, NT, 512).transpose(2, 0, 1, 3)
    )  # [NT, KO, 128, 512]

    in_maps = []
    for c in range(NCORES):
        xs = x[c * TPC : (c + 1) * TPC]  # [TPC, IN_F]
        xTc = np.ascontiguousarray(
            xs.T.reshape(KO, 128, TPC).transpose(1, 0, 2)
        )  # [128, KO, TPC]
        in_maps.append({"xT": _round_f32r(xTc).reshape(128, KO, TPC), "W": W4})
    return in_maps, bias


def _run(x, values, bias, col_indices, trace=False):
    from concourse.bass_utils import run_bass_kernel_spmd

    nc = _build_program()
    in_maps, bias_np = _prep_inputs(x, values, bias, col_indices)
    kwargs = {}
    if trace:
        import tempfile

        kwargs["tmpdir"] = tempfile.mkdtemp(prefix="bass_trace_")
    res = run_bass_kernel_spmd(
        nc, in_maps, list(range(NCORES)), trace=trace, **kwargs
    )
    out = np.concatenate([res.results[c]["out"] for c in range(NCORES)], axis=0)
    if np.any(bias_np):
        out = out + bias_np[None, :]
    return out, res


def kernel(x, values, bias, col_indices):
    out, _ = _run(x, values, bias, col_indices)
    return out


# revision 7
# speedup vs baseline: 1.0052x; 1.0052x over previous
"""Trainium2 kernel for nn_CMSBlockLinear (block-sparse linear layer).

Strategy: the 50%-dense random block topology (16x16 blocks) is hostile to
the 128x128 PE array, so densify the weights host-side (pure index
plumbing, no FLOPs) and run a dense [8192,2048]x[2048,8192] matmul,
token-sharded 8 ways across NeuronCores. All matmul operands use fp32r
(fp32 with 15-bit mantissa) which streams at full PE rate for N>=256.

Per core: out[1024 tok, 8192 feat] = xT_shard.T @ W_dense
  - xT shard [128, 16, 1024] fp32r lives in SBUF; chunks are the
    stationary matmul operand.
  - W streamed in [128, 512] tiles (contiguous 256 KB DMAs), the moving
    operand.
  - psum [128 tok, 512 feat] accumulates over the 16 contraction chunks;
    output is written in [tokens, features] layout directly, so the host
    just concatenates the 8 shards.
"""

import sys

sys.path.insert(0, "/opt/trn_rl_repo")

import numpy as np

T, IN_F, OUT_F = 8192, 2048, 8192
NCORES = 8
TPC = T // NCORES  # 1024 tokens per core
KO = IN_F // 128  # 16 contraction chunks of 128
NT = OUT_F // 512  # 16 feature tiles of 512
MT = TPC // 128  # 8 token tiles of 128

_cached_nc = None


def _round_f32r(a: np.ndarray) -> np.ndarray:
    """Round fp32 to fp32r (15-bit mantissa) with round-to-nearest-even."""
    b = np.ascontiguousarray(a, dtype=np.float32).view(np.uint32)
    r = (b + np.uint32(0x7F) + ((b >> np.uint32(8)) & np.uint32(1))) & np.uint32(
        0xFFFFFF00
    )
    return r.view(np.float32)


def _build_program():
    global _cached_nc
    if _cached_nc is not None:
        return _cached_nc
    from concourse import bacc, mybir, tile

    F32, F32R = mybir.dt.float32, mybir.dt.float32r

    nc = bacc.Bacc(None)
    xT = nc.declare_dram_parameter("xT", [128, KO, TPC], F32R, isOutput=False)
    W = nc.declare_dram_parameter("W", [NT, KO, 128, 512], F32R, isOutput=False)
    out = nc.declare_dram_parameter("out", [TPC, OUT_F], F32, isOutput=True)

    with tile.TileContext(nc) as tc:
        with tc.tile_pool(name="xt", bufs=1) as xpool, \
             tc.tile_pool(name="wt", bufs=12) as wpool, \
             tc.tile_pool(name="ot", bufs=12) as opool, \
             tc.tile_pool(name="ps", bufs=1, space="PSUM") as ps:
            xts = []
            for ko in range(KO):
                # xk0 rides the sync hardware-DGE queue ahead of the W
                # stream: its completion semaphore ticks promptly, so the
                # first matmul isn't gated on the gpsimd software queue's
                # ~4us-late semaphores. The remaining 7.5MB of xT goes on
                # gpsimd so it doesn't steal the W stream's HW queue.
                xk = xpool.tile([128, TPC], F32R, tag=f"x{ko}", name=f"xk{ko}")
                eng = nc.sync if ko == 0 else nc.gpsimd
                eng.dma_start(out=xk[:], in_=xT[:, ko, :])
                xts.append(xk)
            for n in range(NT):
                psums = [
                    ps.tile([128, 512], F32, tag=f"p{m}", name=f"ps{n}_{m}")
                    for m in range(MT)
                ]
                for ko in range(KO):
                    wt = wpool.tile([128, 512], F32R, tag="w", name=f"w{n}_{ko}")
                    nc.sync.dma_start(out=wt[:], in_=W[n, ko])
                    for m in range(MT):
                        nc.tensor.matmul(
                            psums[m][:],
                            xts[ko][:, m * 128 : (m + 1) * 128],
                            wt[:],
                            start=(ko == 0),
                            stop=(ko == KO - 1),
                        )
                for m in range(MT):
                    ot = opool.tile([128, 512], F32, tag="o", name=f"o{n}_{m}")
                    nc.vector.tensor_copy(ot[:], psums[m][:])
                    nc.scalar.dma_start(
                        out=out[m * 128 : (m + 1) * 128, n * 512 : (n + 1) * 512],
                        in_=ot[:],
                    )
    nc.compile()
    _cached_nc = nc
    return nc


def _prep_inputs(x, values, bias, col_indices):
    x = np.ascontiguousarray(np.asarray(x), dtype=np.float32)
    values = np.ascontiguousarray(np.asarray(values), dtype=np.float32)
    bias = np.asarray(bias, dtype=np.float32)
    col_indices = np.asarray(col_indices, dtype=np.int32)

    R, K = col_indices.shape  # 512, 64
    C = IN_F // 16  # 128 column blocks

    # Scatter block values into the dense weight matrix Wd[k_in, n_out].
    Wb = np.zeros((C, R, 16, 16), np.float32)  # [c, r, i, o]
    r_idx = np.broadcast_to(np.arange(R, dtype=np.int64)[:, None], col_indices.shape)
    Wb[col_indices, r_idx] = values.transpose(0, 1, 3, 2)  # values[r,k,o,i] -> [i,o]
    Wd = Wb.transpose(0, 2, 1, 3).reshape(IN_F, OUT_F)
    Wd = _round_f32r(Wd)
    W4 = np.ascontiguousarray(
        Wd.reshape(KO, 128, NT, 512).transpose(2, 0, 1, 3)
    )  # [NT, KO, 128, 512]

    in_maps = []
    for c in range(NCORES):
        xs = x[c * TPC : (c + 1) * TPC]  # [TPC, IN_F]
        xTc = np.ascontiguousarray(
            xs.T.reshape(KO, 128, TPC).transpose(1, 0, 2)
        )  # [128, KO, TPC]
        in_maps.append({"xT": _round_f32r(xTc).reshape(128, KO, TPC), "W": W4})
    return in_maps, bias


def _run(x, values, bias, col_indices, trace=False):
    from concourse.bass_utils import run_bass_kernel_spmd

    nc = _build_program()
    in_maps, bias_np = _prep_inputs(x, values, bias, col_indices)
    kwargs = {}
    if trace:
        import tempfile

        kwargs["tmpdir"] = tempfile.mkdtemp(prefix="bass_trace_")
    res = run_bass_kernel_spmd(
        nc, in_maps, list(range(NCORES)), trace=trace, **kwargs
    )
    out = np.concatenate([res.results[c]["out"] for c in range(NCORES)], axis=0)
    if np.any(bias_np):
        out = out + bias_np[None, :]
    return out, res


def kernel(x, values, bias, col_indices):
    out, _ = _run(x, values, bias, col_indices)
    return out


# revision 8
# speedup vs baseline: 1.0052x; 1.0000x over previous
"""Trainium2 kernel for nn_CMSBlockLinear (block-sparse linear layer).

Strategy: the 50%-dense random block topology (16x16 blocks) is hostile to
the 128x128 PE array, so densify the weights host-side (pure index
plumbing, no FLOPs) and run a dense [8192,2048]x[2048,8192] matmul,
token-sharded 8 ways across NeuronCores. All matmul operands use fp32r
(fp32 with 15-bit mantissa) which streams at full PE rate for N>=256.

Per core: out[1024 tok, 8192 feat] = xT_shard.T @ W_dense
  - xT shard [128, 16, 1024] fp32r lives in SBUF; chunks are the
    stationary matmul operand.
  - W streamed in [128, 512] tiles (contiguous 256 KB DMAs), the moving
    operand.
  - psum [128 tok, 512 feat] accumulates over the 16 contraction chunks;
    output is written in [tokens, features] layout directly, so the host
    just concatenates the 8 shards.
"""

import sys

sys.path.insert(0, "/opt/trn_rl_repo")

import numpy as np

T, IN_F, OUT_F = 8192, 2048, 8192
NCORES = 8
TPC = T // NCORES  # 1024 tokens per core
KO = IN_F // 128  # 16 contraction chunks of 128
NT = OUT_F // 512  # 16 feature tiles of 512
MT = TPC // 128  # 8 token tiles of 128

_cached_nc = None


def _round_f32r(a: np.ndarray) -> np.ndarray:
    """Round fp32 to fp32r (15-bit mantissa) with round-to-nearest-even."""
    b = np.ascontiguousarray(a, dtype=np.float32).view(np.uint32)
    r = (b + np.uint32(0x7F) + ((b >> np.uint32(8)) & np.uint32(1))) & np.uint32(
        0xFFFFFF00
    )
    return r.view(np.float32)


def _build_program():
    global _cached_nc
    if _cached_nc is not None:
        return _cached_nc
    from concourse import bacc, mybir, tile

    F32, F32R = mybir.dt.float32, mybir.dt.float32r

    nc = bacc.Bacc(None)
    xT = nc.declare_dram_parameter("xT", [128, KO, TPC], F32R, isOutput=False)
    W = nc.declare_dram_parameter("W", [NT, KO, 128, 512], F32R, isOutput=False)
    out = nc.declare_dram_parameter("out", [TPC, OUT_F], F32, isOutput=True)

    with tile.TileContext(nc) as tc:
        with tc.tile_pool(name="xt", bufs=1) as xpool, \
             tc.tile_pool(name="wt", bufs=12) as wpool, \
             tc.tile_pool(name="ot", bufs=12) as opool, \
             tc.tile_pool(name="ps", bufs=1, space="PSUM") as ps:
            xts = []
            for ko in range(KO):
                # gpsimd DMA queue: runs in parallel with the W stream on
                # sync, so the first matmuls aren't stuck behind the 8MB
                # xT load sharing one queue. (Measured best: scalar-HW for
                # xT contends with W on the shared AXI port; putting xk0
                # on sync delays W(0,0) — both net-worse.)
                xk = xpool.tile([128, TPC], F32R, tag=f"x{ko}", name=f"xk{ko}")
                nc.gpsimd.dma_start(out=xk[:], in_=xT[:, ko, :])
                xts.append(xk)
            for n in range(NT):
                psums = [
                    ps.tile([128, 512], F32, tag=f"p{m}", name=f"ps{n}_{m}")
                    for m in range(MT)
                ]
                for ko in range(KO):
                    wt = wpool.tile([128, 512], F32R, tag="w", name=f"w{n}_{ko}")
                    nc.sync.dma_start(out=wt[:], in_=W[n, ko])
                    for m in range(MT):
                        nc.tensor.matmul(
                            psums[m][:],
                            xts[ko][:, m * 128 : (m + 1) * 128],
                            wt[:],
                            start=(ko == 0),
                            stop=(ko == KO - 1),
                        )
                for m in range(MT):
                    ot = opool.tile([128, 512], F32, tag="o", name=f"o{n}_{m}")
                    nc.vector.tensor_copy(ot[:], psums[m][:])
                    nc.scalar.dma_start(
                        out=out[m * 128 : (m + 1) * 128, n * 512 : (n + 1) * 512],
                        in_=ot[:],
                    )
    nc.compile()
    _cached_nc = nc
    return nc


def _prep_inputs(x, values, bias, col_indices):
    x = np.ascontiguousarray(np.asarray(x), dtype=np.float32)
    values = np.ascontiguousarray(np.asarray(values), dtype=np.float32)
    bias = np.asarray(bias, dtype=np.float32)
    col_indices = np.asarray(col_indices, dtype=np.int32)

    R, K = col_indices.shape  # 512, 64
    C = IN_F // 16  # 128 column blocks

    # Scatter block values into the dense weight matrix Wd[k_in, n_out].
    Wb = np.zeros((C, R, 16, 16), np.float32)  # [c, r, i, o]
    r_idx = np.broadcast_to(np.arange(R, dtype=np.int64)[:, None], col_indices.shape)
    Wb[col_indices, r_idx] = values.transpose(0, 1, 3, 2)  # values[r,k,o,i] -> [i,o]
    Wd = Wb.transpose(0, 2, 1, 3).reshape(IN_F, OUT_F)
    Wd = _round_f32r(Wd)
    W4 = np.ascontiguousarray(
        Wd.reshape(KO, 128, NT, 512).transpose(2, 0, 1, 3)
    )  # [NT, KO, 128, 512]

    in_maps = []
    for c in range(NCORES):
        xs = x[c * TPC : (c + 1) * TPC]  # [TPC, IN_F]
        xTc = np.ascontiguousarray(
            xs.T.reshape(KO, 128, TPC).transpose(1, 0, 2)
        )  # [128, KO, TPC]
        in_maps.append({"xT": _round_f32r(xTc).reshape(128, KO, TPC), "W": W4})
    return in_maps, bias


def _run(x, values, bias, col_indices, trace=False):
    from concourse.bass_utils import run_bass_kernel_spmd

    nc = _build_program()
    in_maps, bias_np = _prep_inputs(x, values, bias, col_indices)
    kwargs = {}
    if trace:
        import tempfile

        kwargs["tmpdir"] = tempfile.mkdtemp(prefix="bass_trace_")
    res = run_bass_kernel_spmd(
        nc, in_maps, list(range(NCORES)), trace=trace, **kwargs
    )
    out = np.concatenate([res.results[c]["out"] for c in range(NCORES)], axis=0)
    if np.any(bias_np):
        out = out + bias_np[None, :]
    return out, res


def kernel(x, values, bias, col_indices):
    out, _ = _run(x, values, bias, col_indices)
    return out


# revision 9
# speedup vs baseline: 1.0060x; 1.0008x over previous
"""Trainium2 kernel for nn_CMSBlockLinear (block-sparse linear layer).

Strategy: the 50%-dense random block topology (16x16 blocks) is hostile to
the 128x128 PE array, so densify the weights host-side (pure index
plumbing, no FLOPs) and run a dense [8192,2048]x[2048,8192] matmul,
token-sharded 8 ways across NeuronCores. All matmul operands use fp32r
(fp32 with 15-bit mantissa) which streams at full PE rate for N>=256.

Per core: out[1024 tok, 8192 feat] = xT_shard.T @ W_dense
  - xT shard [128, 16, 1024] fp32r lives in SBUF; chunks are the
    stationary matmul operand.
  - W streamed in [128, 512] tiles (contiguous 256 KB DMAs), the moving
    operand.
  - psum [128 tok, 512 feat] accumulates over the 16 contraction chunks;
    output is written in [tokens, features] layout directly, so the host
    just concatenates the 8 shards.
"""

import sys

sys.path.insert(0, "/opt/trn_rl_repo")

import numpy as np

T, IN_F, OUT_F = 8192, 2048, 8192
NCORES = 8
TPC = T // NCORES  # 1024 tokens per core
KO = IN_F // 128  # 16 contraction chunks of 128
NT = OUT_F // 512  # 16 feature tiles of 512
MT = TPC // 128  # 8 token tiles of 128

_cached_nc = None


def _round_f32r(a: np.ndarray) -> np.ndarray:
    """Round fp32 to fp32r (15-bit mantissa) with round-to-nearest-even."""
    b = np.ascontiguousarray(a, dtype=np.float32).view(np.uint32)
    r = (b + np.uint32(0x7F) + ((b >> np.uint32(8)) & np.uint32(1))) & np.uint32(
        0xFFFFFF00
    )
    return r.view(np.float32)


def _build_program():
    global _cached_nc
    if _cached_nc is not None:
        return _cached_nc
    from concourse import bacc, mybir, tile

    F32, F32R = mybir.dt.float32, mybir.dt.float32r

    nc = bacc.Bacc(None)
    xT = nc.declare_dram_parameter("xT", [128, KO, TPC], F32R, isOutput=False)
    W = nc.declare_dram_parameter("W", [NT, KO, 128, 512], F32R, isOutput=False)
    out = nc.declare_dram_parameter("out", [TPC, OUT_F], F32, isOutput=True)

    with tile.TileContext(nc) as tc:
        with tc.tile_pool(name="xt", bufs=1) as xpool, \
             tc.tile_pool(name="wt", bufs=12) as wpool, \
             tc.tile_pool(name="ot", bufs=12) as opool, \
             tc.tile_pool(name="ps", bufs=1, space="PSUM") as ps:
            xts = []
            for ko in range(KO):
                # gpsimd DMA queue: runs in parallel with the W stream on
                # sync, so the first matmuls aren't stuck behind the 8MB
                # xT load sharing one queue. (Measured best: scalar-HW for
                # xT contends with W on the shared AXI port; putting xk0
                # on sync delays W(0,0) — both net-worse.)
                xk = xpool.tile([128, TPC], F32R, tag=f"x{ko}", name=f"xk{ko}")
                nc.gpsimd.dma_start(out=xk[:], in_=xT[:, ko, :])
                xts.append(xk)
            for n in range(NT):
                psums = [
                    ps.tile([128, 512], F32, tag=f"p{m}", name=f"ps{n}_{m}")
                    for m in range(MT)
                ]
                for ko in range(KO):
                    wt = wpool.tile([128, 512], F32R, tag="w", name=f"w{n}_{ko}")
                    nc.sync.dma_start(out=wt[:], in_=W[n, ko])
                    for m in range(MT):
                        nc.tensor.matmul(
                            psums[m][:],
                            xts[ko][:, m * 128 : (m + 1) * 128],
                            wt[:],
                            start=(ko == 0),
                            stop=(ko == KO - 1),
                        )
                for m in range(MT):
                    ot = opool.tile([128, 512], F32, tag="o", name=f"o{n}_{m}")
                    nc.vector.tensor_copy(ot[:], psums[m][:])
                    nc.scalar.dma_start(
                        out=out[m * 128 : (m + 1) * 128, n * 512 : (n + 1) * 512],
                        in_=ot[:],
                    )
    nc.compile()
    _cached_nc = nc
    return nc


def _prep_inputs(x, values, bias, col_indices):
    x = np.ascontiguousarray(np.asarray(x), dtype=np.float32)
    values = np.ascontiguousarray(np.asarray(values), dtype=np.float32)
    bias = np.asarray(bias, dtype=np.float32)
    col_indices = np.asarray(col_indices, dtype=np.int32)

    R, K = col_indices.shape  # 512, 64
    C = IN_F // 16  # 128 column blocks

    # Scatter block values into the dense weight matrix Wd[k_in, n_out].
    Wb = np.zeros((C, R, 16, 16), np.float32)  # [c, r, i, o]
    r_idx = np.broadcast_to(np.arange(R, dtype=np.int64)[:, None], col_indices.shape)
    Wb[col_indices, r_idx] = values.transpose(0, 1, 3, 2)  # values[r,k,o,i] -> [i,o]
    Wd = Wb.transpose(0, 2, 1, 3).reshape(IN_F, OUT_F)
    Wd = _round_f32r(Wd)
    W4 = np.ascontiguousarray(
        Wd.reshape(KO, 128, NT, 512).transpose(2, 0, 1, 3)
    )  # [NT, KO, 128, 512]

    in_maps = []
    for c in range(NCORES):
        xs = x[c * TPC : (c + 1) * TPC]  # [TPC, IN_F]
        xTc = np.ascontiguousarray(
            xs.T.reshape(KO, 128, TPC).transpose(1, 0, 2)
        )  # [128, KO, TPC]
        in_maps.append({"xT": _round_f32r(xTc).reshape(128, KO, TPC), "W": W4})
    return in_maps, bias


def _run(x, values, bias, col_indices, trace=False):
    from concourse.bass_utils import run_bass_kernel_spmd

    nc = _build_program()
    in_maps, bias_np = _prep_inputs(x, values, bias, col_indices)
    kwargs = {}
    if trace:
        import tempfile

        kwargs["tmpdir"] = tempfile.mkdtemp(prefix="bass_trace_")
    try:
        res = run_bass_kernel_spmd(
            nc, in_maps, list(range(NCORES)), trace=trace, **kwargs
        )
    except Exception:
        # Transient device wedges (NRT_EXEC_UNIT_UNRECOVERABLE) have been
        # observed to clear on retry.
        import time

        time.sleep(20)
        res = run_bass_kernel_spmd(
            nc, in_maps, list(range(NCORES)), trace=trace, **kwargs
        )
    out = np.concatenate([res.results[c]["out"] for c in range(NCORES)], axis=0)
    if np.any(bias_np):
        out = out + bias_np[None, :]
    return out, res


def kernel(x, values, bias, col_indices):
    out, _ = _run(x, values, bias, col_indices)
    return out


# revision 10
# speedup vs baseline: 1.0090x; 1.0030x over previous
"""Trainium2 kernel for nn_CMSBlockLinear (block-sparse linear layer).

Strategy: the 50%-dense random block topology (16x16 blocks) is hostile to
the 128x128 PE array, so densify the weights host-side (pure index
plumbing, no FLOPs) and run a dense [8192,2048]x[2048,8192] matmul,
token-sharded 8 ways across NeuronCores. All matmul operands use fp32r
(fp32 with 15-bit mantissa) which streams at full PE rate for N>=256.

Per core: out[1024 tok, 8192 feat] = xT_shard.T @ W_dense
  - xT shard [128, 16, 1024] fp32r lives in SBUF; chunks are the
    stationary matmul operand.
  - W streamed in [128, 512] tiles (contiguous 256 KB DMAs), the moving
    operand.
  - psum [128 tok, 512 feat] accumulates over the 16 contraction chunks;
    output is written in [tokens, features] layout directly, so the host
    just concatenates the 8 shards.
"""

import sys

sys.path.insert(0, "/opt/trn_rl_repo")

import numpy as np

T, IN_F, OUT_F = 8192, 2048, 8192
NCORES = 8
TPC = T // NCORES  # 1024 tokens per core
KO = IN_F // 128  # 16 contraction chunks of 128
NT = OUT_F // 512  # 16 feature tiles of 512
MT = TPC // 128  # 8 token tiles of 128

_cached_nc = None


def _round_f32r(a: np.ndarray) -> np.ndarray:
    """Round fp32 to fp32r (15-bit mantissa) with round-to-nearest-even."""
    b = np.ascontiguousarray(a, dtype=np.float32).view(np.uint32)
    r = (b + np.uint32(0x7F) + ((b >> np.uint32(8)) & np.uint32(1))) & np.uint32(
        0xFFFFFF00
    )
    return r.view(np.float32)


def _build_program():
    global _cached_nc
    if _cached_nc is not None:
        return _cached_nc
    from concourse import bacc, mybir, tile

    F32, F32R = mybir.dt.float32, mybir.dt.float32r

    nc = bacc.Bacc(None)
    xT = nc.declare_dram_parameter("xT", [128, KO, TPC], F32R, isOutput=False)
    W = nc.declare_dram_parameter("W", [NT, KO, 128, 512], F32R, isOutput=False)
    out = nc.declare_dram_parameter("out", [TPC, OUT_F], F32, isOutput=True)

    with tile.TileContext(nc) as tc:
        with tc.tile_pool(name="xt", bufs=1) as xpool, \
             tc.tile_pool(name="wt", bufs=12) as wpool, \
             tc.tile_pool(name="ot", bufs=12) as opool, \
             tc.tile_pool(name="ps", bufs=1, space="PSUM") as ps:
            # HAM pre-warm: the PE sits idle ~6us while the first DMAs
            # land, and its clock gate only reaches 2.4GHz after ~3.4us of
            # sustained activity. Dummy matmuls on a zeroed tile fill the
            # idle window so the real matmuls start at full clock. They
            # write a psum slot (tag p7) whose first real use is ~1.5us
            # after the warmup drains, and start=True on the real matmul
            # overwrites whatever the warmup left there.
            wz = xpool.tile([128, 512], F32, tag="warmf", name="warm_f32")
            nc.vector.memset(wz[:], 0.0)
            warm = xpool.tile([128, 512], F32R, tag="warmr", name="warm_r")
            nc.vector.tensor_copy(warm[:], wz[:])
            wps = ps.tile([128, 512], F32, tag=f"p{MT-1}", name="warm_ps")
            for i in range(14):
                nc.tensor.matmul(wps[:], warm[:, :128], warm[:], start=True, stop=True)

            xts = []
            for ko in range(KO):
                # gpsimd DMA queue: runs in parallel with the W stream on
                # sync, so the first matmuls aren't stuck behind the 8MB
                # xT load sharing one queue. (Measured best: scalar-HW for
                # xT contends with W on the shared AXI port; putting xk0
                # on sync delays W(0,0) — both net-worse.)
                xk = xpool.tile([128, TPC], F32R, tag=f"x{ko}", name=f"xk{ko}")
                nc.gpsimd.dma_start(out=xk[:], in_=xT[:, ko, :])
                xts.append(xk)
            for n in range(NT):
                psums = [
                    ps.tile([128, 512], F32, tag=f"p{m}", name=f"ps{n}_{m}")
                    for m in range(MT)
                ]
                for ko in range(KO):
                    wt = wpool.tile([128, 512], F32R, tag="w", name=f"w{n}_{ko}")
                    nc.sync.dma_start(out=wt[:], in_=W[n, ko])
                    for m in range(MT):
                        nc.tensor.matmul(
                            psums[m][:],
                            xts[ko][:, m * 128 : (m + 1) * 128],
                            wt[:],
                            start=(ko == 0),
                            stop=(ko == KO - 1),
                        )
                for m in range(MT):
                    ot = opool.tile([128, 512], F32, tag="o", name=f"o{n}_{m}")
                    nc.vector.tensor_copy(ot[:], psums[m][:])
                    nc.scalar.dma_start(
                        out=out[m * 128 : (m + 1) * 128, n * 512 : (n + 1) * 512],
                        in_=ot[:],
                    )
    nc.compile()
    _cached_nc = nc
    return nc


def _prep_inputs(x, values, bias, col_indices):
    x = np.ascontiguousarray(np.asarray(x), dtype=np.float32)
    values = np.ascontiguousarray(np.asarray(values), dtype=np.float32)
    bias = np.asarray(bias, dtype=np.float32)
    col_indices = np.asarray(col_indices, dtype=np.int32)

    R, K = col_indices.shape  # 512, 64
    C = IN_F // 16  # 128 column blocks

    # Scatter block values into the dense weight matrix Wd[k_in, n_out].
    Wb = np.zeros((C, R, 16, 16), np.float32)  # [c, r, i, o]
    r_idx = np.broadcast_to(np.arange(R, dtype=np.int64)[:, None], col_indices.shape)
    Wb[col_indices, r_idx] = values.transpose(0, 1, 3, 2)  # values[r,k,o,i] -> [i,o]
    Wd = Wb.transpose(0, 2, 1, 3).reshape(IN_F, OUT_F)
    Wd = _round_f32r(Wd)
    W4 = np.ascontiguousarray(
        Wd.reshape(KO, 128, NT, 512).transpose(2, 0, 1, 3)
    )  # [NT, KO, 128, 512]

    in_maps = []
    for c in range(NCORES):
        xs = x[c * TPC : (c + 1) * TPC]  # [TPC, IN_F]
        xTc = np.ascontiguousarray(
            xs.T.reshape(KO, 128, TPC).transpose(1, 0, 2)
        )  # [128, KO, TPC]
        in_maps.append({"xT": _round_f32r(xTc).reshape(128, KO, TPC), "W": W4})
    return in_maps, bias


def _run(x, values, bias, col_indices, trace=False):
    from concourse.bass_utils import run_bass_kernel_spmd

    nc = _build_program()
    in_maps, bias_np = _prep_inputs(x, values, bias, col_indices)
    kwargs = {}
    if trace:
        import tempfile

        kwargs["tmpdir"] = tempfile.mkdtemp(prefix="bass_trace_")
    try:
        res = run_bass_kernel_spmd(
            nc, in_maps, list(range(NCORES)), trace=trace, **kwargs
        )
    except Exception:
        # Transient device wedges (NRT_EXEC_UNIT_UNRECOVERABLE) have been
        # observed to clear on retry.
        import time

        time.sleep(20)
        res = run_bass_kernel_spmd(
            nc, in_maps, list(range(NCORES)), trace=trace, **kwargs
        )
    out = np.concatenate([res.results[c]["out"] for c in range(NCORES)], axis=0)
    if np.any(bias_np):
        out = out + bias_np[None, :]
    return out, res


def kernel(x, values, bias, col_indices):
    out, _ = _run(x, values, bias, col_indices)
    return out
